# revision 17
# baseline (speedup 1.0000x reference)
"""Trainium2 Bass kernel for nn_CrossEntropyLoss_2585570312585 (v3).

Reference:
    cw = where(cw == 0, cw[0], cw)                      # [5]
    gold2dim   = argmax(gold, axis=class)               # [256,384]
    prediction = argmax(pred, axis=class)
    pred_fp    = where(gold2dim > 0, 0, prediction)
    loss = -(weight + cw[pred_fp]) * sum_c(gold * log(pred + 1e-8))
    out  = mean(loss)

Measured constraints driving the design (this toolchain/HW):
  * Fixed NEFF pre/postamble ~10.9 us (empty-program floor); first DMA
    issue can't start before ~6.7 us, teardown ~2.9 us after last op.
  * DMA DRAM->SBUF is row(descriptor)-bound: ~11-20 ns/row depending on
    row bytes (240B->11, 960B->19); partition-offset destinations are
    3-10x pathological.  3 DGE queues (SP, Activation, Pool) run in
    parallel.  => 3 full-128-row DMAs, one per queue; data lands ~9.8.
  * DVE: reduces ~1.1 ns/elem regardless of dtype; tensor_tensor gets
    bf16 2x ONLY when both operands are contiguous (broadcast kills it).
  * GpSimd elementwise is 2.5x slower and stalls DVE (verified) ->
    all compute on DVE; ACT does ln + the m-broadcast copy.
  * tensor_tensor_reduce / activation(accum_out) crash at runtime
    (verified) -> plain ops only.
  * bf16 inputs: offline exact simulation vs the deterministic
    reference inputs gives rel_err 4.1e-3 (budget 2e-2).

Algebra:
  gold2dim>0  <=>  max_all(gold) > gold[0]
  S2-factorization: sum_jc eq*cw*vu = sum_j vu_j * (sum_c eq_jc cw_c)
  single-output fold: total = sum_j u_j * q_j,
      q = w + gmask*cw0 + (1-gmask)*zc,   zc = sum_c eq_jc cw_c
  host: loss = -total / N.

Device layout per core (12288 px as [128, 96(j), 5(c)] class-minor):
  pg tile [128, 1920] u8 = gold bf16 [128,480] | pred bf16 [128,480]
     (adjacent so ONE reduce computes [mg | m])
  aw tile [128, 576] u8 = g0 contig bf16 [128,96] | weight f32 [128,96]
  mmg   = max-reduce [128,192,5] -> [mg | m] bf16       (DVE 1062)
  gmask = mg > g0c                                       (DVE  170)
  L     = ln(pred + eps)           (ACT, bf16)
  m480  = broadcast m over classes (ACT copy, bf16)
  prod  = gold * L -> uz[:, :480]                        (DVE  319)
  eq    = (pred == m480)   contiguous bf16               (DVE  319)
  z1    = eq * cwb -> uz[:, 480:]                        (DVE  318)
  uzr   = add-reduce [128,192,5] -> [u | zc] f32         (DVE 1062)
  bv    = [gmask*cw0 + w | (gmask-1)*u]                  (DVE  430)
  bz    = bv * [u | zc] ; acc[:,0:2] = sum_j bz          (DVE  710)
  out: [128,6] f32 DMA (24B rows; narrower rows blow up the final
       barrier: 8B rows -> +2us, 4B rows -> +6.5us, measured)
Host: loss = -(sum acc0 - sum acc1) / 98304
"""

import os
import sys

import numpy as np
import ml_dtypes


def _ensure_concourse():
    try:
        import concourse  # noqa: F401
        return
    except ImportError:
        pass
    for p in ("/opt/trn_rl_repo", "/root/.axon_site/_ro/trn_rl_repo"):
        if os.path.isdir(p) and p not in sys.path:
            sys.path.insert(0, p)
    import concourse  # noqa: F401


_ensure_concourse()

import concourse.bass as bass  # noqa: E402
import concourse.tile as tile  # noqa: E402
from concourse import bacc, mybir  # noqa: E402
from concourse.bass_utils import run_bass_kernel_spmd  # noqa: E402

N_CORES = 8
H, W = 256, 384
N_PIX = H * W                      # 98304
PIX_PER_CORE = N_PIX // N_CORES    # 12288
P = 128
F = PIX_PER_CORE // P              # 96
C = 5
EPS = 1e-8
BF = ml_dtypes.bfloat16

F32 = mybir.dt.float32
BF16 = mybir.dt.bfloat16
U8 = mybir.dt.uint8
Alu = mybir.AluOpType
ActFn = mybir.ActivationFunctionType
AxX = mybir.AxisListType.X

TRACE = False
LAST_RESULTS = None

_PROGRAM_CACHE = {}


def _build_program(cw_adj):
    cw0 = float(cw_adj[0])
    nc = bacc.Bacc(
        "TRN2",
        target_bir_lowering=False,
        debug=False,
        enable_asserts=False,
        num_devices=N_CORES,
    )

    pg_d = nc.dram_tensor("pg", [P, 4 * C * F], U8, kind="ExternalInput").ap()
    aw_d = nc.dram_tensor("aw", [P, 6 * F], U8, kind="ExternalInput").ap()
    # 24-byte rows: narrower out-DMA rows (4B/8B) blow the final barrier
    # up to 5-9.5us (measured); 24B rows keep it at ~3us.
    acc_d = nc.dram_tensor("acc", [P, 6], F32, kind="ExternalOutput").ap()

    with tile.TileContext(nc) as tc:
        with tc.tile_pool(name="main", bufs=1) as pool:
            # constants while engines idle
            eps_t = pool.tile([P, 1], F32)
            nc.vector.memset(eps_t[:], EPS)
            wrm_t = pool.tile([P, 1], BF16)
            nc.vector.memset(wrm_t[:], 1.0)
            cwb_t = pool.tile([P, C * F], BF16)
            cwb_jc = cwb_t[:].rearrange("p (j c) -> p j c", c=C)
            for c in range(C):
                nc.vector.memset(cwb_jc[:, :, c], float(cw_adj[c]))
            acc_t = pool.tile([P, 6], F32)
            nc.vector.memset(acc_t[:, 1:6], 0.0)

            # input DMAs: gold||pred adjacent in one tile (column ranges,
            # full 128 partitions each), aux on the Pool queue.  The scalar
            # DMA is emitted BEFORE the Ln warmup: an intervening DMA on
            # the Activation engine splits the activation group and forces
            # a second 1.3us ACT_TABLE_LOAD.
            pg_t = pool.tile([P, 4 * C * F], U8)
            nc.sync.dma_start(out=pg_t[:, 0 : 2 * C * F],
                              in_=pg_d[:, 0 : 2 * C * F])
            nc.scalar.dma_start(out=pg_t[:, 2 * C * F : 4 * C * F],
                                in_=pg_d[:, 2 * C * F : 4 * C * F])
            aw_t = pool.tile([P, 6 * F], U8)
            nc.gpsimd.dma_start(out=aw_t[:], in_=aw_d)

            # warm the Ln table (bf16 in/out, matching the real ln)
            wrm2_t = pool.tile([P, 1], BF16)
            nc.scalar.activation(wrm2_t[:], wrm_t[:], ActFn.Ln, bias=eps_t[:])

            gp = pg_t[:].bitcast(BF16)                    # [128, 960] g|p
            gb = gp[:, 0 : C * F]                         # [128, 480]
            pb = gp[:, C * F : 2 * C * F]
            g0c = aw_t[:, 0 : 2 * F].bitcast(BF16)        # [128, 96]
            w_v = aw_t[:, 2 * F : 6 * F].bitcast(F32)     # [128, 96]

            # [mg | m] in one reduce over [128, 192, 5]
            mmg_t = pool.tile([P, 2 * F], BF16)
            nc.vector.tensor_reduce(
                mmg_t[:], gp.rearrange("p (j c) -> p j c", c=C),
                axis=AxX, op=Alu.max,
            )
            mg_v = mmg_t[:, 0:F]
            m_v = mmg_t[:, F : 2 * F]

            gmask_t = pool.tile([P, F], F32)
            nc.vector.tensor_tensor(gmask_t[:], mg_v, g0c, op=Alu.is_gt)

            # ACT: ln, then broadcast m -> [128,480] (contiguous eq operand)
            L_t = pool.tile([P, C * F], BF16)
            nc.scalar.activation(L_t[:], pb, ActFn.Ln, bias=eps_t[:])
            m480_t = pool.tile([P, C * F], BF16)
            m_b = m_v.unsqueeze(2).broadcast_to([P, F, C])
            nc.scalar.copy(m480_t[:].rearrange("p (j c) -> p j c", c=C), m_b)

            # uz = [gold*L | eq*cwb]
            uz_t = pool.tile([P, 2 * C * F], BF16)
            nc.vector.tensor_tensor(uz_t[:, 0 : C * F], gb, L_t[:],
                                    op=Alu.mult)
            eq_t = pool.tile([P, C * F], BF16)
            nc.vector.tensor_tensor(eq_t[:], pb, m480_t[:], op=Alu.is_equal)
            nc.vector.tensor_tensor(uz_t[:, C * F : 2 * C * F], eq_t[:],
                                    cwb_t[:], op=Alu.mult)

            uzr_t = pool.tile([P, 2 * F], F32)
            nc.vector.tensor_reduce(
                uzr_t[:], uz_t[:].rearrange("p (j c) -> p j c", c=C),
                axis=AxX, op=Alu.add,
            )
            u_v = uzr_t[:, 0:F]
            zc_v = uzr_t[:, F : 2 * F]

            # bv = [gmask*cw0 + w | (gmask-1)*u]; bz = bv * [u | zc]
            # acc0 = sum bz0 = sum base*u ; acc1 = sum bz1 = sum vu*zc
            # host: total = acc0 - acc1
            bv_t = pool.tile([P, 2 * F], F32)
            nc.vector.scalar_tensor_tensor(
                bv_t[:, 0:F], gmask_t[:], cw0, w_v, op0=Alu.mult, op1=Alu.add)
            nc.vector.scalar_tensor_tensor(
                bv_t[:, F : 2 * F], gmask_t[:], 1.0, u_v,
                op0=Alu.subtract, op1=Alu.mult)
            bz_t = pool.tile([P, 2 * F], F32)
            nc.vector.tensor_tensor(bz_t[:], bv_t[:], uzr_t[:], op=Alu.mult)
            nc.vector.tensor_reduce(
                acc_t[:, 0:2], bz_t[:].rearrange("p (k j) -> p k j", j=F),
                axis=AxX, op=Alu.add)

            nc.sync.dma_start(out=acc_d, in_=acc_t[:], single_packet=True)

    nc.compile()
    return nc


def _interleave_bf16(arr5: np.ndarray, core: int) -> np.ndarray:
    chunk = arr5[:, core * PIX_PER_CORE : (core + 1) * PIX_PER_CORE]
    il = chunk.reshape(C, P, F).transpose(1, 2, 0).reshape(P, C * F)
    return np.ascontiguousarray(il.astype(BF)).view(np.uint8)


def kernel(pred, gold, weight, clss_weight_list):
    global LAST_RESULTS

    pred = np.asarray(pred, dtype=np.float32)
    gold = np.asarray(gold, dtype=np.float32)
    weight = np.asarray(weight, dtype=np.float32)
    cw = np.asarray(clss_weight_list, dtype=np.float32)[0]
    cw_adj = np.where(cw == 0, cw[0], cw).astype(np.float32)

    key = cw_adj.tobytes()
    nc = _PROGRAM_CACHE.get(key)
    if nc is None:
        nc = _build_program(cw_adj)
        _PROGRAM_CACHE[key] = nc

    p5 = pred[0].reshape(C, N_PIX)
    g5 = gold[0].reshape(C, N_PIX)
    w1 = weight[0].reshape(N_PIX)

    in_maps = []
    for k in range(N_CORES):
        sl = slice(k * PIX_PER_CORE, (k + 1) * PIX_PER_CORE)
        pg = np.empty((P, 4 * C * F), dtype=np.uint8)
        pg[:, 0 : 2 * C * F] = _interleave_bf16(g5, k)
        pg[:, 2 * C * F :] = _interleave_bf16(p5, k)
        aw = np.empty((P, 6 * F), dtype=np.uint8)
        aw[:, 0 : 2 * F] = g5[0, sl].reshape(P, F).astype(BF).view(np.uint8)
        aw[:, 2 * F :] = np.ascontiguousarray(
            w1[sl].reshape(P, F)).view(np.uint8)
        in_maps.append({"pg": pg, "aw": aw})

    res = run_bass_kernel_spmd(
        nc, in_maps, list(range(N_CORES)), trace=TRACE
    )
    LAST_RESULTS = res

    total = 0.0
    for k in range(N_CORES):
        acc = np.asarray(res.results[k]["acc"], dtype=np.float64)
        total += acc[:, 0].sum() - acc[:, 1].sum()

    loss = -total / N_PIX
    return np.float32(loss)


# revision 18
# speedup vs baseline: 1.1171x; 1.1171x over previous
"""Trainium2 Bass kernel for nn_CrossEntropyLoss_2585570312585.

Reference computation (jax):
    cw = where(cw == 0, cw[0], cw)                      # [5]
    gold2dim   = argmax(gold, axis=class)               # [256,384]
    prediction = argmax(pred, axis=class)
    pred_fp    = where(gold2dim > 0, 0,
                       where(prediction == gold2dim, 0, prediction))
    weight_fp  = cw[pred_fp]
    loss = -(weight + weight_fp) * sum_c(gold * log(pred + 1e-8))
    out  = mean(loss)                                   # scalar

Algebraic restructuring (exactly equivalent up to fp assoc):
  * pred_fp = where(gold2dim > 0, 0, prediction)  -- the inner where is a
    no-op when gold2dim == 0 since prediction == gold2dim implies
    prediction == 0 there.
  * gold2dim > 0  <=>  max(g[1:5]) > g[0]   (exact, incl. argmax ties)
  * cw[prediction] = sum_c cw_c * (p_c == max_c p_c)  (exact except exact
    float ties between classes, which double-count; measure-zero inputs)
  * The scalar mean decomposes into per-class partial sums, so the device
    returns per-partition partials and the host applies cw and the final
    tiny reduction during the gather step.

Sharding: the 256x384 = 98304-pixel plane is split into 8 contiguous
chunks of 12288 pixels (one per NeuronCore), laid out as [128 partitions
x 96 pixels]. The host pre-packs per-core buffers CLASS-MINOR
(interleaved: free index j*5 + c) so every class reduction on DVE is
inner-contiguous (~645 ns vs ~950 ns for strided). gold and weight are
packed into one buffer so each core does two input DMAs total, issued
from different DGEs (SP HWDGE + Pool SWDGE) for parallel descriptor gen.

HW-measured notes driving the design (see session notes):
  * GpSimd elementwise compute contends with DVE on SBUF ports (measured
    2.5x slowdown of concurrent DVE ops) -> all compute on DVE, ACT does
    ln + casts, Pool only issues a DMA.
  * tensor_tensor_reduce / DMA accum / Pool max are rejected or broken on
    this toolchain -> plain mult+reduce forms only.
  * bf16 tensor_tensor gets 2x (400 ns vs 648 ns at [128,480]); used for
    the prod and z products where rounding provably cannot bias the
    result beyond ~1e-5 relative.

Device per core (all tiles [128, 480] interleaved unless noted):
  L    = ln(pred + 1e-8)  -> bf16              (ACT)
  gb   = bf16(gold)                            (ACT copy)
  prod = gb * L           (bf16 2x)            (DVE)
  u    = sum_c prod        -> [128,96] f32     (DVE reduce, contiguous)
  m    = max_c pred        -> [128,96] f32     (DVE reduce, contiguous)
  eq   = (pred == m_bcast) -> bf16             (DVE)
  gr   = max(g1..g4)       -> [128,96]         (DVE reduce, contiguous)
  gmask= gr > g0                               (DVE, g0 stride-5 view)
  vu   = (gmask - 1) * u   -> bf16             (DVE fused stt)
  z    = eq * vu_bcast     (bf16 2x)           (DVE)
  accz = sum_pixels z      -> [128, 5] f32     (DVE reduce, strided)
  base = gmask * cw0 + weight                  (DVE fused stt)
  bu   = base * u ; acc1 = sum_pixels bu       (DVE)
Host: loss = -(sum acc1 - sum_c cw_c * sum accz_c) / 98304
"""

import os
import sys

import numpy as np


def _ensure_concourse():
    try:
        import concourse  # noqa: F401
        return
    except ImportError:
        pass
    for p in ("/opt/trn_rl_repo", "/root/.axon_site/_ro/trn_rl_repo"):
        if os.path.isdir(p) and p not in sys.path:
            sys.path.insert(0, p)
    import concourse  # noqa: F401


_ensure_concourse()

import concourse.bass as bass  # noqa: E402
import concourse.tile as tile  # noqa: E402
from concourse import bacc, mybir  # noqa: E402
from concourse.bass_utils import run_bass_kernel_spmd  # noqa: E402

N_CORES = 8
H, W = 256, 384
N_PIX = H * W                      # 98304
PIX_PER_CORE = N_PIX // N_CORES    # 12288
P = 128                            # partitions
F = PIX_PER_CORE // P              # 96 free-dim pixels per partition
C = 5                              # classes
EPS = 1e-8

F32 = mybir.dt.float32
BF16 = mybir.dt.bfloat16
Alu = mybir.AluOpType
ActFn = mybir.ActivationFunctionType
AxX = mybir.AxisListType.X

# Set by callers that want a profile; results stashed in LAST_RESULTS.
TRACE = False
LAST_RESULTS = None

_PROGRAM_CACHE = {}


def _build_program(cw0: float):
    """Build + compile the per-core Bass program (same program on all 8
    cores; only the data differs). cw0 is baked as an immediate."""
    nc = bacc.Bacc(
        "TRN2",
        target_bir_lowering=False,
        debug=False,
        enable_asserts=False,
        num_devices=N_CORES,
    )

    # pred: [128, 480] interleaved (j*5 + c); goldw: gold interleaved 480
    # cols then weight 96 cols.
    pred_d = nc.dram_tensor("pred", [P, C * F], F32, kind="ExternalInput").ap()
    goldw_d = nc.dram_tensor(
        "goldw", [P, C * F + F], F32, kind="ExternalInput"
    ).ap()
    acc_d = nc.dram_tensor("acc", [P, 6], F32, kind="ExternalOutput").ap()

    with tile.TileContext(nc) as tc:
        with tc.tile_pool(name="main", bufs=1) as pool:
            # eps bias tile for ln(p + eps)
            eps_t = pool.tile([P, 1], F32)
            nc.vector.memset(eps_t[:], EPS)

            # Warm up the ACT ln table before the input DMAs land.
            warm = pool.tile([P, 1], F32)
            nc.vector.memset(warm[:], 1.0)
            nc.scalar.activation(warm[:], warm[:], ActFn.Ln, bias=eps_t[:])

            p_t = pool.tile([P, C * F], F32)
            nc.sync.dma_start(out=p_t[:], in_=pred_d)
            gw_t = pool.tile([P, C * F + F], F32)
            nc.gpsimd.dma_start(out=gw_t[:], in_=goldw_d)

            # interleaved views: [128, 96(j), 5(c)], inner (class) stride 1
            p_jc = p_t[:].rearrange("p (j c) -> p j c", c=C)
            g_jc = gw_t[:, 0 : C * F].rearrange("p (j c) -> p j c", c=C)
            w_v = gw_t[:, C * F : C * F + F]

            # L = ln(pred + eps), bf16 out
            L_t = pool.tile([P, C * F], BF16)
            nc.scalar.activation(L_t[:], p_t[:], ActFn.Ln, bias=eps_t[:])

            # gb = bf16(gold) on ACT (idle; keeps DVE free)
            gb_t = pool.tile([P, C * F], BF16)
            nc.scalar.copy(gb_t[:], gw_t[:, 0 : C * F])

            # prod = gb * L (bf16 2x)
            prod_t = pool.tile([P, C * F], BF16)
            nc.vector.tensor_tensor(prod_t[:], gb_t[:], L_t[:], op=Alu.mult)

            # u = sum_c prod  [128,96] f32 (inner-contiguous reduce)
            u_t = pool.tile([P, F], F32)
            nc.vector.tensor_reduce(
                u_t[:], prod_t[:].rearrange("p (j c) -> p j c", c=C),
                axis=AxX, op=Alu.add,
            )

            # m = max_c pred  [128,96]
            m_t = pool.tile([P, F], F32)
            nc.vector.tensor_reduce(m_t[:], p_jc, axis=AxX, op=Alu.max)

            # eq = (pred == m) -> bf16, interleaved layout
            eq_t = pool.tile([P, C * F], BF16)
            eq_jc = eq_t[:].rearrange("p (j c) -> p j c", c=C)
            m_b = m_t[:].unsqueeze(2).broadcast_to([P, F, C])
            nc.vector.tensor_tensor(eq_jc, p_jc, m_b, op=Alu.is_equal)

            # gr = max(g1..g4) (inner-contiguous, offset 1)
            gr_t = pool.tile([P, F], F32)
            nc.vector.tensor_reduce(
                gr_t[:], g_jc[:, :, 1:5], axis=AxX, op=Alu.max
            )

            # gmask = gr > g0 (g0 is the stride-5 class-0 view)
            gmask_t = pool.tile([P, F], F32)
            nc.vector.tensor_tensor(
                gmask_t[:], gr_t[:], g_jc[:, :, 0], op=Alu.is_gt
            )

            # vu = (gmask - 1) * u -> bf16
            vu_t = pool.tile([P, F], BF16)
            nc.vector.scalar_tensor_tensor(
                vu_t[:], gmask_t[:], 1.0, u_t[:],
                op0=Alu.subtract, op1=Alu.mult,
            )

            # z = eq * vu (bf16 2x), interleaved
            z_t = pool.tile([P, C * F], BF16)
            z_jc = z_t[:].rearrange("p (j c) -> p j c", c=C)
            vu_b = vu_t[:].unsqueeze(2).broadcast_to([P, F, C])
            nc.vector.tensor_tensor(z_jc, eq_jc, vu_b, op=Alu.mult)

            # accumulator tile: col0 = acc1, cols 1..5 = accz
            acc_t = pool.tile([P, 6], F32)
            # accz_c = sum_j z[j, c]  (strided reduce over j)
            z_cj = z_t[:].rearrange("p (j c) -> p c j", c=C)
            nc.vector.tensor_reduce(acc_t[:, 1:6], z_cj, axis=AxX, op=Alu.add)

            # base = gmask * cw0 + w
            base_t = pool.tile([P, F], F32)
            nc.vector.scalar_tensor_tensor(
                base_t[:], gmask_t[:], float(cw0), w_v,
                op0=Alu.mult, op1=Alu.add,
            )

            # acc1 = sum_pixels base * u
            bu_t = pool.tile([P, F], F32)
            nc.vector.tensor_tensor(bu_t[:], base_t[:], u_t[:], op=Alu.mult)
            nc.vector.tensor_reduce(acc_t[:, 0:1], bu_t[:], axis=AxX, op=Alu.add)

            nc.sync.dma_start(out=acc_d, in_=acc_t[:])

    nc.compile()
    return nc


def _interleave(arr5: np.ndarray, core: int) -> np.ndarray:
    """arr5: [5, 98304] -> per-core [128, 480] class-minor (free index
    j*5 + c)."""
    chunk = arr5[:, core * PIX_PER_CORE : (core + 1) * PIX_PER_CORE]
    # [5, 128, 96] -> [128, 96, 5] -> [128, 480]
    return chunk.reshape(C, P, F).transpose(1, 2, 0).reshape(P, C * F)


def kernel(pred, gold, weight, clss_weight_list):
    global LAST_RESULTS

    pred = np.asarray(pred, dtype=np.float32)
    gold = np.asarray(gold, dtype=np.float32)
    weight = np.asarray(weight, dtype=np.float32)
    cw = np.asarray(clss_weight_list, dtype=np.float32)[0]  # [5]
    cw_adj = np.where(cw == 0, cw[0], cw).astype(np.float32)
    cw0 = float(cw_adj[0])

    key = np.float32(cw0).tobytes()
    nc = _PROGRAM_CACHE.get(key)
    if nc is None:
        nc = _build_program(cw0)
        _PROGRAM_CACHE[key] = nc

    p5 = pred[0].reshape(C, N_PIX)
    g5 = gold[0].reshape(C, N_PIX)
    w1 = weight[0].reshape(N_PIX)

    in_maps = []
    for k in range(N_CORES):
        gw = np.empty((P, C * F + F), dtype=np.float32)
        gw[:, 0 : C * F] = _interleave(g5, k)
        gw[:, C * F :] = w1[k * PIX_PER_CORE : (k + 1) * PIX_PER_CORE].reshape(
            P, F
        )
        in_maps.append(
            {
                "pred": np.ascontiguousarray(_interleave(p5, k)),
                "goldw": gw,
            }
        )

    res = run_bass_kernel_spmd(
        nc, in_maps, list(range(N_CORES)), trace=TRACE
    )
    LAST_RESULTS = res

    total = 0.0
    cw64 = cw_adj.astype(np.float64)
    for k in range(N_CORES):
        acc = np.asarray(res.results[k]["acc"], dtype=np.float64)  # [128,6]
        total += acc[:, 0].sum()
        total -= (cw64 * acc[:, 1:6].sum(axis=0)).sum()

    loss = -total / N_PIX
    return np.float32(loss)



# revision 19
# speedup vs baseline: 1.1188x; 1.0016x over previous
"""Trainium2 Bass kernel for nn_CrossEntropyLoss_2585570312585 (v3).

Reference:
    cw = where(cw == 0, cw[0], cw)                      # [5]
    gold2dim   = argmax(gold, axis=class)               # [256,384]
    prediction = argmax(pred, axis=class)
    pred_fp    = where(gold2dim > 0, 0, prediction)
    loss = -(weight + cw[pred_fp]) * sum_c(gold * log(pred + 1e-8))
    out  = mean(loss)

Measured constraints driving the design (this toolchain/HW):
  * Fixed NEFF pre/postamble ~10.9 us (empty-program floor); first DMA
    issue can't start before ~6.7 us, teardown ~2.9 us after last op.
  * DMA DRAM->SBUF is row(descriptor)-bound: ~11-20 ns/row depending on
    row bytes (240B->11, 960B->19); partition-offset destinations are
    3-10x pathological.  3 DGE queues (SP, Activation, Pool) run in
    parallel.  => 3 full-128-row DMAs, one per queue; data lands ~9.8.
  * DVE: reduces ~1.1 ns/elem regardless of dtype; tensor_tensor gets
    bf16 2x ONLY when both operands are contiguous (broadcast kills it).
  * GpSimd elementwise is 2.5x slower and stalls DVE (verified) ->
    all compute on DVE; ACT does ln + the m-broadcast copy.
  * tensor_tensor_reduce / activation(accum_out) crash at runtime
    (verified) -> plain ops only.
  * bf16 inputs: offline exact simulation vs the deterministic
    reference inputs gives rel_err 4.1e-3 (budget 2e-2).

Algebra:
  gold2dim>0  <=>  max_all(gold) > gold[0]
  S2-factorization: sum_jc eq*cw*vu = sum_j vu_j * (sum_c eq_jc cw_c)
  single-output fold: total = sum_j u_j * q_j,
      q = w + gmask*cw0 + (1-gmask)*zc,   zc = sum_c eq_jc cw_c
  host: loss = -total / N.

Device layout per core (12288 px as [128, 96(j), 5(c)] class-minor):
  pg tile [128, 1920] u8 = gold bf16 [128,480] | pred bf16 [128,480]
     (adjacent so ONE reduce computes [mg | m])
  aw tile [128, 576] u8 = g0 contig bf16 [128,96] | weight f32 [128,96]
  mmg   = max-reduce [128,192,5] -> [mg | m] bf16       (DVE 1062)
  gmask = mg > g0c                                       (DVE  170)
  L     = ln(pred + eps)           (ACT, bf16)
  m480  = broadcast m over classes (ACT copy, bf16)
  prod  = gold * L -> uz[:, :480]                        (DVE  319)
  eq    = (pred == m480)   contiguous bf16               (DVE  319)
  z1    = eq * cwb -> uz[:, 480:]                        (DVE  318)
  uzr   = add-reduce [128,192,5] -> [u | zc] f32         (DVE 1062)
  bv    = [gmask*cw0 + w | (gmask-1)*u]                  (DVE  430)
  bz    = bv * [u | zc] ; acc[:,0:2] = sum_j bz          (DVE  710)
  out: [128,6] f32 DMA (24B rows; narrower rows blow up the final
       barrier: 8B rows -> +2us, 4B rows -> +6.5us, measured)
Host: loss = -(sum acc0 - sum acc1) / 98304
"""

import os
import sys

import numpy as np
import ml_dtypes


def _ensure_concourse():
    try:
        import concourse  # noqa: F401
        return
    except ImportError:
        pass
    for p in ("/opt/trn_rl_repo", "/root/.axon_site/_ro/trn_rl_repo"):
        if os.path.isdir(p) and p not in sys.path:
            sys.path.insert(0, p)
    import concourse  # noqa: F401


_ensure_concourse()

import concourse.bass as bass  # noqa: E402
import concourse.tile as tile  # noqa: E402
from concourse import bacc, mybir  # noqa: E402
from concourse.bass_utils import run_bass_kernel_spmd  # noqa: E402

N_CORES = 8
H, W = 256, 384
N_PIX = H * W                      # 98304
PIX_PER_CORE = N_PIX // N_CORES    # 12288
P = 128
F = PIX_PER_CORE // P              # 96
C = 5
EPS = 1e-8
BF = ml_dtypes.bfloat16

F32 = mybir.dt.float32
BF16 = mybir.dt.bfloat16
U8 = mybir.dt.uint8
Alu = mybir.AluOpType
ActFn = mybir.ActivationFunctionType
AxX = mybir.AxisListType.X

TRACE = False
LAST_RESULTS = None

_PROGRAM_CACHE = {}


def _build_program(cw_adj):
    cw0 = float(cw_adj[0])
    nc = bacc.Bacc(
        "TRN2",
        target_bir_lowering=False,
        debug=False,
        enable_asserts=False,
        num_devices=N_CORES,
    )

    pg_d = nc.dram_tensor("pg", [P, 4 * C * F], U8, kind="ExternalInput").ap()
    aw_d = nc.dram_tensor("aw", [P, 6 * F], U8, kind="ExternalInput").ap()
    # 24-byte rows: narrower out-DMA rows (4B/8B) blow the final barrier
    # up to 5-9.5us (measured); 24B rows keep it at ~3us.
    acc_d = nc.dram_tensor("acc", [P, 6], F32, kind="ExternalOutput").ap()

    with tile.TileContext(nc) as tc:
        with tc.tile_pool(name="main", bufs=1) as pool:
            # constants while engines idle
            eps_t = pool.tile([P, 1], F32)
            nc.vector.memset(eps_t[:], EPS)
            wrm_t = pool.tile([P, 1], BF16)
            nc.vector.memset(wrm_t[:], 1.0)
            cwb_t = pool.tile([P, C * F], BF16)
            cwb_jc = cwb_t[:].rearrange("p (j c) -> p j c", c=C)
            for c in range(C):
                nc.vector.memset(cwb_jc[:, :, c], float(cw_adj[c]))
            acc_t = pool.tile([P, 6], F32)
            nc.vector.memset(acc_t[:, 1:6], 0.0)

            # input DMAs: gold||pred adjacent in one tile (column ranges,
            # full 128 partitions each), aux on the Pool queue.  The scalar
            # DMA is emitted BEFORE the Ln warmup: an intervening DMA on
            # the Activation engine splits the activation group and forces
            # a second 1.3us ACT_TABLE_LOAD.
            pg_t = pool.tile([P, 4 * C * F], U8)
            nc.sync.dma_start(out=pg_t[:, 0 : 2 * C * F],
                              in_=pg_d[:, 0 : 2 * C * F])
            nc.scalar.dma_start(out=pg_t[:, 2 * C * F : 4 * C * F],
                                in_=pg_d[:, 2 * C * F : 4 * C * F])
            aw_t = pool.tile([P, 6 * F], U8)
            nc.gpsimd.dma_start(out=aw_t[:], in_=aw_d)

            # warm the Ln table (bf16 in/out, matching the real ln)
            wrm2_t = pool.tile([P, 1], BF16)
            nc.scalar.activation(wrm2_t[:], wrm_t[:], ActFn.Ln, bias=eps_t[:])

            gp = pg_t[:].bitcast(BF16)                    # [128, 960] g|p
            gb = gp[:, 0 : C * F]                         # [128, 480]
            pb = gp[:, C * F : 2 * C * F]
            g0c = aw_t[:, 0 : 2 * F].bitcast(BF16)        # [128, 96]
            w_v = aw_t[:, 2 * F : 6 * F].bitcast(F32)     # [128, 96]

            # [mg | m] in one reduce over [128, 192, 5]
            mmg_t = pool.tile([P, 2 * F], BF16)
            nc.vector.tensor_reduce(
                mmg_t[:], gp.rearrange("p (j c) -> p j c", c=C),
                axis=AxX, op=Alu.max,
            )
            mg_v = mmg_t[:, 0:F]
            m_v = mmg_t[:, F : 2 * F]

            gmask_t = pool.tile([P, F], F32)
            nc.vector.tensor_tensor(gmask_t[:], mg_v, g0c, op=Alu.is_gt)

            # ACT: ln, then broadcast m -> [128,480] (contiguous eq operand)
            L_t = pool.tile([P, C * F], BF16)
            nc.scalar.activation(L_t[:], pb, ActFn.Ln, bias=eps_t[:])
            m480_t = pool.tile([P, C * F], BF16)
            m_b = m_v.unsqueeze(2).broadcast_to([P, F, C])
            nc.scalar.copy(m480_t[:].rearrange("p (j c) -> p j c", c=C), m_b)

            # uz = [gold*L | eq*cwb]
            uz_t = pool.tile([P, 2 * C * F], BF16)
            nc.vector.tensor_tensor(uz_t[:, 0 : C * F], gb, L_t[:],
                                    op=Alu.mult)
            eq_t = pool.tile([P, C * F], BF16)
            nc.vector.tensor_tensor(eq_t[:], pb, m480_t[:], op=Alu.is_equal)
            nc.vector.tensor_tensor(uz_t[:, C * F : 2 * C * F], eq_t[:],
                                    cwb_t[:], op=Alu.mult)

            uzr_t = pool.tile([P, 2 * F], F32)
            nc.vector.tensor_reduce(
                uzr_t[:], uz_t[:].rearrange("p (j c) -> p j c", c=C),
                axis=AxX, op=Alu.add,
            )
            u_v = uzr_t[:, 0:F]
            zc_v = uzr_t[:, F : 2 * F]

            # bv = [gmask*cw0 + w | (gmask-1)*u]; bz = bv * [u | zc]
            # acc0 = sum bz0 = sum base*u ; acc1 = sum bz1 = sum vu*zc
            # host: total = acc0 - acc1
            bv_t = pool.tile([P, 2 * F], F32)
            nc.vector.scalar_tensor_tensor(
                bv_t[:, 0:F], gmask_t[:], cw0, w_v, op0=Alu.mult, op1=Alu.add)
            nc.vector.scalar_tensor_tensor(
                bv_t[:, F : 2 * F], gmask_t[:], 1.0, u_v,
                op0=Alu.subtract, op1=Alu.mult)
            bz_t = pool.tile([P, 2 * F], F32)
            nc.vector.tensor_tensor(bz_t[:], bv_t[:], uzr_t[:], op=Alu.mult)
            nc.vector.tensor_reduce(
                acc_t[:, 0:2], bz_t[:].rearrange("p (k j) -> p k j", j=F),
                axis=AxX, op=Alu.add)

            nc.sync.dma_start(out=acc_d, in_=acc_t[:], single_packet=True)

    nc.compile()
    return nc


def _interleave_bf16(arr5: np.ndarray, core: int) -> np.ndarray:
    chunk = arr5[:, core * PIX_PER_CORE : (core + 1) * PIX_PER_CORE]
    il = chunk.reshape(C, P, F).transpose(1, 2, 0).reshape(P, C * F)
    return np.ascontiguousarray(il.astype(BF)).view(np.uint8)


def kernel(pred, gold, weight, clss_weight_list):
    global LAST_RESULTS

    pred = np.asarray(pred, dtype=np.float32)
    gold = np.asarray(gold, dtype=np.float32)
    weight = np.asarray(weight, dtype=np.float32)
    cw = np.asarray(clss_weight_list, dtype=np.float32)[0]
    cw_adj = np.where(cw == 0, cw[0], cw).astype(np.float32)

    key = cw_adj.tobytes()
    nc = _PROGRAM_CACHE.get(key)
    if nc is None:
        nc = _build_program(cw_adj)
        _PROGRAM_CACHE[key] = nc

    p5 = pred[0].reshape(C, N_PIX)
    g5 = gold[0].reshape(C, N_PIX)
    w1 = weight[0].reshape(N_PIX)

    in_maps = []
    for k in range(N_CORES):
        sl = slice(k * PIX_PER_CORE, (k + 1) * PIX_PER_CORE)
        pg = np.empty((P, 4 * C * F), dtype=np.uint8)
        pg[:, 0 : 2 * C * F] = _interleave_bf16(g5, k)
        pg[:, 2 * C * F :] = _interleave_bf16(p5, k)
        aw = np.empty((P, 6 * F), dtype=np.uint8)
        aw[:, 0 : 2 * F] = g5[0, sl].reshape(P, F).astype(BF).view(np.uint8)
        aw[:, 2 * F :] = np.ascontiguousarray(
            w1[sl].reshape(P, F)).view(np.uint8)
        in_maps.append({"pg": pg, "aw": aw})

    res = run_bass_kernel_spmd(
        nc, in_maps, list(range(N_CORES)), trace=TRACE
    )
    LAST_RESULTS = res

    total = 0.0
    for k in range(N_CORES):
        acc = np.asarray(res.results[k]["acc"], dtype=np.float64)
        total += acc[:, 0].sum() - acc[:, 1].sum()

    loss = -total / N_PIX
    return np.float32(loss)
